# revision 6
# baseline (speedup 1.0000x reference)
"""Trainium2 Bass kernel for nn_MixDensityLoss.

Data-parallel over batch: each of the 8 cores handles 2 batch elements.
The alignment DP (logaddexp recursion over T=800 steps) is restructured as
an L-column sweep: per column, a Viterbi max-plus scan provides a per-cell
normalizer V, and the exact logsumexp recursion becomes a linear first-order
recurrence in normalized exp space (state = p*state + f*Aprev, with p,f <= 1
by construction), executed with the DVE tensor_tensor_scan instruction.
alpha = V + ln(Ahat). Serial chain: 2 instructions per column per sweep.
"""
import numpy as np
from contextlib import ExitStack

import concourse.bass as bass
import concourse.bacc as bacc
import concourse.tile as tile
import concourse.mybir as mybir
from concourse.bass_utils import run_bass_kernel_spmd

F32 = mybir.dt.float32
AX = mybir.AluOpType
AF = mybir.ActivationFunctionType

B, T, D, L = 16, 800, 80, 160
NCORES = 8
BPC = B // NCORES          # 2 batch elements per core
NEG = -1.0e20
CAP = 1.0e21
EPS = 1.0e-15
LN_SCALE = 2.0 ** -17
LN_OFF = 17 * 0.6931471805599453
DLOG2PI = D * 1.8378770664093453  # D * log(2*pi)
CHUNK = 4                  # sweep columns per SBUF chunk
NCHUNK = L // CHUNK
HL = L // 2                # 80, l-half size
TT = [(0, 128), (128, 128), (256, 128), (384, 128),
      (512, 128), (640, 128), (768, 32)]

_cache = {}


def build(debug=False):
    nc = bacc.Bacc("TRN2", target_bir_lowering=False, debug=False)

    mel_d = nc.dram_tensor("mel", [BPC, T, D], F32, kind="ExternalInput")
    ms_d = nc.dram_tensor("mu_sigma", [BPC, L, 2 * D], F32, kind="ExternalInput")
    ident_d = nc.dram_tensor("ident", [128, 128], F32, kind="ExternalInput")
    onesrow_d = nc.dram_tensor("onesrow", [1, 128], F32, kind="ExternalInput")
    onescol_d = nc.dram_tensor("onescol", [128, 1], F32, kind="ExternalInput")

    lp_out = nc.dram_tensor("log_prob", [BPC, T, L], F32, kind="ExternalOutput")
    al_out = nc.dram_tensor("alphas", [BPC, T, L], F32, kind="ExternalOutput")

    skind = "ExternalOutput" if debug else "Internal"
    V_d = nc.dram_tensor("V_scr", [BPC, L, T + 1], F32, kind=skind)
    d1_d = nc.dram_tensor("d1_scr", [BPC, L, T], F32, kind=skind)
    p_d = nc.dram_tensor("p_scr", [BPC, L, T], F32, kind=skind)
    f_d = nc.dram_tensor("f_scr", [BPC, L, T], F32, kind=skind)
    A_d = nc.dram_tensor("A_scr", [BPC, L, T + 1], F32, kind=skind)

    with tile.TileContext(nc) as tc:
        with ExitStack() as ctx:
            persist = ctx.enter_context(tc.tile_pool(name="persist", bufs=1))

            ident = persist.tile([128, 128], F32)
            nc.sync.dma_start(ident[:], ident_d[:])
            onesrow = persist.tile([1, 128], F32)
            nc.sync.dma_start(onesrow[:], onesrow_d[:])
            onescol = persist.tile([128, 1], F32)
            nc.sync.dma_start(onescol[:], onescol_d[:])
            consts = persist.tile([128, 2], F32)
            nc.vector.memset(consts[:, 0:1], DLOG2PI)
            nc.vector.memset(consts[:, 1:2], EPS)

            # ---------------- phase 1: log_prob ----------------
            melT, mel2T, ivv, miv2, rrow = [], [], [], [], []
            lpT = []  # lpT[b][h] : [80, T] l-major log_prob
            with ExitStack() as c1:
                p1 = c1.enter_context(tc.tile_pool(name="p1", bufs=1))
                ld = c1.enter_context(tc.tile_pool(name="p1ld", bufs=3))
                ps = c1.enter_context(tc.tile_pool(name="p1ps", bufs=2, space="PSUM"))

                for b in range(BPC):
                    # mu_sigma -> muT, lsT in [D, L] layout
                    muT = p1.tile([D, L], F32, tag="muT")
                    lsT = p1.tile([D, L], F32, tag="lsT")
                    for h in range(2):
                        msl = ld.tile([HL, 2 * D], F32, tag="msl")
                        nc.sync.dma_start(msl[:], ms_d[b, h * HL:(h + 1) * HL, :])
                        for dh, dst in ((0, muT), (1, lsT)):
                            tr = ps.tile([D, HL], F32, tag="trm", bufs=1)
                            nc.tensor.transpose(
                                tr[:], msl[:, dh * D:(dh + 1) * D], ident[0:HL, 0:HL])
                            nc.scalar.copy(dst[:, h * HL:(h + 1) * HL], tr[:])
                    sig = p1.tile([D, L], F32, tag="sig")
                    nc.scalar.activation(sig[:], muT[:], AF.Sigmoid)
                    mu4 = p1.tile([D, L], F32, tag="mu4")
                    nc.vector.tensor_scalar_mul(mu4[:], sig[:], 4.0)
                    th = p1.tile([D, L], F32, tag="th")
                    nc.scalar.activation(th[:], lsT[:], AF.Tanh)
                    ls4 = p1.tile([D, L], F32, tag="ls4")
                    nc.vector.tensor_scalar_mul(ls4[:], th[:], 4.0)
                    iv = p1.tile([D, L], F32, tag=f"iv{b}")
                    nc.scalar.activation(iv[:], ls4[:], AF.Exp, scale=-2.0)
                    mm = p1.tile([D, L], F32, tag="mm")
                    nc.vector.tensor_mul(mm[:], mu4[:], iv[:])
                    m2 = p1.tile([D, L], F32, tag=f"m2{b}")
                    nc.vector.tensor_scalar_mul(m2[:], mm[:], -2.0)
                    rr = p1.tile([D, L], F32, tag="rr")
                    nc.vector.tensor_mul(rr[:], mu4[:], mm[:])
                    rr2 = p1.tile([D, L], F32, tag="rr2")
                    nc.vector.tensor_add(rr2[:], rr[:], ls4[:])
                    psr = ps.tile([1, L], F32, tag="psr", bufs=1)
                    nc.tensor.matmul(psr[:], onescol[0:D, 0:1], rr2[:],
                                     start=True, stop=True)
                    rw = p1.tile([1, L], F32, tag=f"rw{b}")
                    nc.scalar.activation(rw[:], psr[:], AF.Identity,
                                         bias=consts[0:1, 0:1])
                    ivv.append(iv); miv2.append(m2); rrow.append(rw)

                    # mel -> melT [D, T], mel2T
                    mT = p1.tile([D, T], F32, tag=f"melT{b}")
                    for (t0, tt) in TT:
                        mload = ld.tile([128, D], F32, tag="mload")
                        nc.sync.dma_start(mload[0:tt, :], mel_d[b, t0:t0 + tt, :])
                        trp = ps.tile([D, 128], F32, tag="trmel")
                        nc.tensor.transpose(trp[0:D, 0:tt], mload[0:tt, :],
                                            ident[0:tt, 0:tt])
                        nc.scalar.copy(mT[:, t0:t0 + tt], trp[0:D, 0:tt])
                    m2T = p1.tile([D, T], F32, tag=f"mel2T{b}")
                    nc.scalar.activation(m2T[:], mT[:], AF.Square)
                    melT.append(mT); mel2T.append(m2T)

                # matmuls -> lp tiles; transposes -> lpT
                for b in range(BPC):
                    lpT.append([persist.tile([HL, T], F32, tag=f"lpT{b}{h}", name=f"lpT{b}{h}")
                                for h in range(2)])
                for (t0, tt) in TT:
                    psl = ps.tile([128, BPC * L], F32, tag="psl")
                    for b in range(BPC):
                        sl = psl[0:tt, b * L:(b + 1) * L]
                        nc.tensor.matmul(sl, mel2T[b][:, t0:t0 + tt], ivv[b][:],
                                         start=True, stop=False)
                        nc.tensor.matmul(sl, melT[b][:, t0:t0 + tt], miv2[b][:],
                                         start=False, stop=False)
                        nc.tensor.matmul(sl, onesrow[0:1, 0:tt], rrow[b][:],
                                         start=False, stop=True)
                    lpt = ld.tile([128, BPC * L], F32, tag="lpt")
                    nc.scalar.mul(lpt[0:tt, :], psl[0:tt, :], -0.5)
                    for b in range(BPC):
                        nc.sync.dma_start(lp_out[b, t0:t0 + tt, :],
                                          lpt[0:tt, b * L:(b + 1) * L])
                        for h in range(2):
                            trp = ps.tile([HL, 128], F32, tag="trlp")
                            nc.tensor.transpose(
                                trp[0:HL, 0:tt],
                                lpt[0:tt, b * L + h * HL: b * L + (h + 1) * HL],
                                ident[0:tt, 0:tt])
                            nc.scalar.copy(lpT[b][h][:, t0:t0 + tt],
                                           trp[0:HL, 0:tt])

            # ---------------- V sweep ----------------
            W = T + 1  # column slot width (guard at 0)
            with ExitStack() as c2:
                sp = c2.enter_context(tc.tile_pool(name="vsweep", bufs=1))
                vbuf = [sp.tile([BPC, CHUNK * W], F32, tag=f"vb{i}", name=f"vb{i}")
                        for i in range(2)]
                d1buf = [sp.tile([BPC, CHUNK * T], F32, tag=f"db{i}", name=f"db{i}")
                         for i in range(2)]
                lpbuf = [sp.tile([BPC, CHUNK * T], F32, tag=f"lb{i}", name=f"lb{i}")
                         for i in range(2)]
                nc.vector.memset(vbuf[0][:], NEG)
                nc.vector.memset(vbuf[1][:], NEG)

                for c in range(NCHUNK):
                    l0 = c * CHUNK
                    h, lo = divmod(l0, HL)
                    vb, db, lb = vbuf[c % 2], d1buf[c % 2], lpbuf[c % 2]
                    pvb = vbuf[1 - c % 2]
                    for b in range(BPC):
                        nc.sync.dma_start(lb[b:b + 1, :],
                                          lpT[b][h][lo:lo + CHUNK, :])
                    for j in range(CHUNK):
                        l = l0 + j
                        lpc = lb[:, j * T:(j + 1) * T]
                        prev = (vb[:, (j - 1) * W:(j - 1) * W + T] if j > 0
                                else pvb[:, (CHUNK - 1) * W:(CHUNK - 1) * W + T])
                        d1c = db[:, j * T:(j + 1) * T]
                        nc.vector.tensor_add(d1c, lpc, prev)
                        nc.vector.tensor_tensor_scan(
                            vb[:, j * W + 1:(j + 1) * W], lpc, d1c,
                            0.0 if l == 0 else NEG, op0=AX.add, op1=AX.max)
                    for b in range(BPC):
                        nc.sync.dma_start(V_d[b, l0:l0 + CHUNK, :],
                                          vb[b:b + 1, :])
                        nc.sync.dma_start(d1_d[b, l0:l0 + CHUNK, :],
                                          db[b:b + 1, :])

            # ---------------- bulk p / f ----------------
            with ExitStack() as c3:
                bp = c3.enter_context(tc.tile_pool(name="bulk", bufs=2))
                for b in range(BPC):
                    for h in range(2):
                        vt = bp.tile([HL, W], F32, tag="vt")
                        nc.sync.dma_start(vt[:], V_d[b, h * HL:(h + 1) * HL, :])
                        d1t = bp.tile([HL, T], F32, tag="d1t")
                        nc.sync.dma_start(d1t[:], d1_d[b, h * HL:(h + 1) * HL, :])
                        q = bp.tile([HL, T], F32, tag="q")
                        nc.vector.tensor_add(q[:], lpT[b][h][:], vt[:, 0:T])
                        parg = bp.tile([HL, T], F32, tag="parg")
                        nc.vector.tensor_sub(parg[:], q[:], vt[:, 1:W])
                        pt = bp.tile([HL, T], F32, tag="pt")
                        nc.scalar.activation(pt[:], parg[:], AF.Exp)
                        nc.vector.memset(pt[:, 0:1], 1.0)
                        farg = bp.tile([HL, T], F32, tag="farg")
                        nc.vector.tensor_sub(farg[:], d1t[:], vt[:, 1:W])
                        ft = bp.tile([HL, T], F32, tag="ft")
                        nc.scalar.activation(ft[:], farg[:], AF.Exp)
                        nc.sync.dma_start(p_d[b, h * HL:(h + 1) * HL, :], pt[:])
                        nc.sync.dma_start(f_d[b, h * HL:(h + 1) * HL, :], ft[:])

            # ---------------- A sweep ----------------
            with ExitStack() as c4:
                sp = c4.enter_context(tc.tile_pool(name="asweep", bufs=1))
                abuf = [sp.tile([BPC, CHUNK * W], F32, tag=f"ab{i}", name=f"ab{i}")
                        for i in range(2)]
                dabuf = [sp.tile([BPC, CHUNK * T], F32, tag=f"dab{i}", name=f"dab{i}")
                         for i in range(2)]
                pbuf = [sp.tile([BPC, CHUNK * T], F32, tag=f"pb{i}", name=f"pb{i}")
                        for i in range(2)]
                fbuf = [sp.tile([BPC, CHUNK * T], F32, tag=f"fb{i}", name=f"fb{i}")
                        for i in range(2)]
                nc.vector.memset(abuf[0][:], 0.0)
                nc.vector.memset(abuf[1][:], 0.0)

                for c in range(NCHUNK):
                    l0 = c * CHUNK
                    ab, dab = abuf[c % 2], dabuf[c % 2]
                    pb, fb = pbuf[c % 2], fbuf[c % 2]
                    pab = abuf[1 - c % 2]
                    for b in range(BPC):
                        nc.sync.dma_start(pb[b:b + 1, :],
                                          p_d[b, l0:l0 + CHUNK, :])
                        nc.sync.dma_start(fb[b:b + 1, :],
                                          f_d[b, l0:l0 + CHUNK, :])
                    for j in range(CHUNK):
                        pc = pb[:, j * T:(j + 1) * T]
                        fc = fb[:, j * T:(j + 1) * T]
                        aprev = (ab[:, (j - 1) * W:(j - 1) * W + T] if j > 0
                                 else pab[:, (CHUNK - 1) * W:(CHUNK - 1) * W + T])
                        dac = dab[:, j * T:(j + 1) * T]
                        nc.vector.scalar_tensor_tensor(
                            dac, aprev, CAP, fc, op0=AX.min, op1=AX.mult)
                        nc.vector.tensor_tensor_scan(
                            ab[:, j * W + 1:(j + 1) * W], pc, dac, 1.0,
                            op0=AX.mult, op1=AX.add)
                    for b in range(BPC):
                        nc.sync.dma_start(A_d[b, l0:l0 + CHUNK, :],
                                          ab[b:b + 1, :])

            # ---------------- recovery: alpha = V + ln(A + eps) ----------------
            with ExitStack() as c5:
                rp = c5.enter_context(tc.tile_pool(name="rec", bufs=2))
                rps = c5.enter_context(tc.tile_pool(name="recps", bufs=4,
                                                    space="PSUM"))
                for b in range(BPC):
                    alT = []
                    for h in range(2):
                        at = rp.tile([HL, W], F32, tag="at")
                        nc.sync.dma_start(at[:], A_d[b, h * HL:(h + 1) * HL, :])
                        vt = rp.tile([HL, W], F32, tag="vt2")
                        nc.sync.dma_start(vt[:], V_d[b, h * HL:(h + 1) * HL, :])
                        ln = rp.tile([HL, T], F32, tag="ln")
                        nc.scalar.activation(ln[:], at[:, 1:W], AF.Ln,
                                             bias=consts[0:HL, 1:2],
                                             scale=LN_SCALE)
                        alt = rp.tile([HL, T], F32, tag=f"alt{h}")
                        nc.vector.scalar_tensor_tensor(
                            alt[:], ln[:], LN_OFF, vt[:, 1:W],
                            op0=AX.add, op1=AX.add)
                        alT.append(alt)
                    for (t0, tt) in TT:
                        ot = rp.tile([128, L], F32, tag="ot")
                        for h in range(2):
                            trp = rps.tile([128, HL], F32, tag="trr")
                            nc.tensor.transpose(trp[0:tt, 0:HL],
                                                alT[h][:, t0:t0 + tt],
                                                ident[0:HL, 0:HL])
                            nc.scalar.copy(ot[0:tt, h * HL:(h + 1) * HL],
                                           trp[0:tt, 0:HL])
                        nc.sync.dma_start(al_out[b, t0:t0 + tt, :], ot[0:tt, :])

    nc.compile()
    return nc


def _get_nc():
    if "nc" not in _cache:
        _cache["nc"] = build()
    return _cache["nc"]


def kernel(mel, mu_sigma, mel_lengths, character_lengths):
    mel = np.ascontiguousarray(np.asarray(mel, dtype=np.float32))
    mu_sigma = np.ascontiguousarray(np.asarray(mu_sigma, dtype=np.float32))
    ml = np.asarray(mel_lengths).astype(np.int64)
    cl = np.asarray(character_lengths).astype(np.int64)

    nc = _get_nc()
    ident = np.eye(128, dtype=np.float32)
    onesrow = np.ones((1, 128), dtype=np.float32)
    onescol = np.ones((128, 1), dtype=np.float32)
    in_maps = []
    for c in range(NCORES):
        sl = slice(BPC * c, BPC * (c + 1))
        in_maps.append({
            "mel": np.ascontiguousarray(mel[sl]),
            "mu_sigma": np.ascontiguousarray(mu_sigma[sl]),
            "ident": ident, "onesrow": onesrow, "onescol": onescol,
        })
    res = run_bass_kernel_spmd(nc, in_maps, list(range(NCORES))).results
    log_prob = np.concatenate([res[c]["log_prob"] for c in range(NCORES)], axis=0)
    alphas = np.concatenate([res[c]["alphas"] for c in range(NCORES)], axis=0)
    mm = int(ml.max())
    if mm < T:
        alphas[:, mm:, :] = 0.0
    last = alphas[np.arange(B), ml - 1, cl - 1]
    loss = np.float32(-last.mean())
    return log_prob, loss, alphas
